# revision 1
# baseline (speedup 1.0000x reference)
"""LocalPatchAttention Trainium2 kernel.

Data-parallel over batch B=8 across 8 NeuronCores (one image per core).
Per-core pipeline (all channel counts hardcoded for the B,Cq,H,W = 8,64,256,256 /
Cv,h,w = 128,64,64 problem):

  - q rows stream in [64ch, 512px] pairs (2 image rows).
  - PE-transpose 128-px chunks -> [128px, 64ch] in PSUM; LayerNorm stats via
    bn_stats/bn_aggr on VectorE (free-dim reduce); normalize with a 2-op
    tensor_scalar ((x-mu)*rsqrt) writing bf16.
  - PE-transpose back to [64ch, 128px]; one matmul with the host-prefolded
    [64,128] matrix A = scale * (g*qW^T) @ K^T gives the attention logits;
    Sigmoid on ScalarE with the folded bias as per-partition bias.
  - x_attn = sig * V via stride-0 broadcast APs (V per 4x4 patch cell), V
    precomputed once per core with the same LN/linear folding.
  - 3x3 conv = 12 PSUM-accumulated matmuls per 4 output rows, output-channel
    dim packed 2 rows deep (M=128); conv bias folded in as a K=1 matmul;
    fp32 residual add with the resident q rows; stream out.
"""

import numpy as np
import ml_dtypes

import concourse.bass as bass
import concourse.bacc as bacc
import concourse.tile as tile
from concourse import mybir
from concourse.bass_utils import run_bass_kernel_spmd

F32 = mybir.dt.float32
BF16 = mybir.dt.bfloat16
AF = mybir.ActivationFunctionType
ALU = mybir.AluOpType
EPS = 1e-5
NPBF16 = ml_dtypes.bfloat16

_CACHE = {}


def _build_nc():
    nc = bacc.Bacc()
    q_d = nc.declare_dram_parameter("q", [64, 65536], F32, isOutput=False)
    v_d = nc.declare_dram_parameter("v", [128, 4096], F32, isOutput=False)
    A_d = nc.declare_dram_parameter("Amat", [64, 128], BF16, isOutput=False)
    cb_d = nc.declare_dram_parameter("cbias", [128, 1], F32, isOutput=False)
    vwf_d = nc.declare_dram_parameter("vwf", [128, 128], BF16, isOutput=False)
    vbp_d = nc.declare_dram_parameter("vbp", [128, 1], F32, isOutput=False)
    cwt_d = nc.declare_dram_parameter("cwt", [128, 1536], BF16, isOutput=False)
    cbb_d = nc.declare_dram_parameter("cbb", [1, 128], BF16, isOutput=False)
    i64_d = nc.declare_dram_parameter("i64", [64, 64], BF16, isOutput=False)
    i128_d = nc.declare_dram_parameter("i128", [128, 128], BF16, isOutput=False)
    out_d = nc.declare_dram_parameter("out", [64, 65536], F32, isOutput=True)

    with tile.TileContext(nc) as tc, \
         tc.tile_pool(name="const", bufs=1) as cpool, \
         tc.tile_pool(name="vwork", bufs=1) as vpool, \
         tc.tile_pool(name="qin", bufs=6) as qin_pool, \
         tc.tile_pool(name="qb", bufs=4) as qb_pool, \
         tc.tile_pool(name="xh", bufs=8) as xh_pool, \
         tc.tile_pool(name="xhT", bufs=3) as xhT_pool, \
         tc.tile_pool(name="sig", bufs=4) as sig_pool, \
         tc.tile_pool(name="srow", bufs=16) as srow_pool, \
         tc.tile_pool(name="stat", bufs=8) as st_pool, \
         tc.tile_pool(name="outp", bufs=3) as out_pool, \
         tc.tile_pool(name="ps_tp", bufs=4, space="PSUM") as ps_tp, \
         tc.tile_pool(name="ps_lg", bufs=2, space="PSUM") as ps_lg, \
         tc.tile_pool(name="ps_cv", bufs=2, space="PSUM") as ps_cv:

        def const_tile(shape, dtype, tag, src):
            t = cpool.tile(shape, dtype, tag=tag)
            nc.sync.dma_start(out=t, in_=src[:, :])
            return t

        A_sb = const_tile([64, 128], BF16, "A", A_d)
        cb_sb = const_tile([128, 1], F32, "cb", cb_d)
        vwf_sb = const_tile([128, 128], BF16, "vwf", vwf_d)
        vbp_sb = const_tile([128, 1], F32, "vbp", vbp_d)
        cwt_sb = const_tile([128, 1536], BF16, "cwt", cwt_d)
        cbb_sb = const_tile([1, 128], BF16, "cbb", cbb_d)
        i64_sb = const_tile([64, 64], BF16, "i64", i64_d)
        i128_sb = const_tile([128, 128], BF16, "i128", i128_d)

        ones512b = cpool.tile([1, 512], BF16, tag="o512")
        nc.vector.memset(ones512b, 1.0)
        ones128f = cpool.tile([128, 1], F32, tag="o128")
        nc.vector.memset(ones128f, 1.0)
        ones1x128 = cpool.tile([1, 128], F32, tag="o1x")
        nc.vector.memset(ones1x128, 1.0)
        zrow = cpool.tile([128, 256], BF16, tag="zr")
        nc.vector.memset(zrow, 0.0)

        # ---------------- V path (once per core) ----------------
        vraw = vpool.tile([128, 4096], F32, tag="vraw")
        vsq = vpool.tile([128, 4096], F32, tag="vsq")
        vhat = vpool.tile([128, 4096], BF16, tag="vhat")
        V_sb = vpool.tile([128, 4096], F32, tag="V")
        for ch in range(8):
            sl = slice(ch * 512, (ch + 1) * 512)
            nc.sync.dma_start(out=vraw[:, sl], in_=v_d[:, sl])
            nc.scalar.activation(vsq[:, sl], vraw[:, sl], AF.Square)
            s_ps = ps_tp.tile([1, 512], F32, tag="t")
            nc.tensor.matmul(s_ps, ones128f, vraw[:, sl], start=True, stop=True)
            sq_ps = ps_tp.tile([1, 512], F32, tag="t")
            nc.tensor.matmul(sq_ps, ones128f, vsq[:, sl], start=True, stop=True)
            mu = st_pool.tile([1, 512], F32, tag="vmu")
            nc.vector.tensor_scalar_mul(mu, s_ps, 1.0 / 128)
            var = st_pool.tile([1, 512], F32, tag="vvar")
            nc.vector.tensor_mul(var, mu, mu)
            msq = st_pool.tile([1, 512], F32, tag="vmsq")
            nc.vector.tensor_scalar(msq, sq_ps, 1.0 / 128, None, ALU.mult)
            nc.vector.tensor_sub(var, msq, var)
            nc.vector.tensor_scalar_add(var, var, EPS)
            rec = st_pool.tile([1, 512], F32, tag="vrec")
            nc.vector.reciprocal(rec, var)
            rr = st_pool.tile([1, 512], F32, tag="vr")
            nc.scalar.activation(rr, rec, AF.Sqrt)
            rb = ps_lg.tile([128, 512], F32, tag="lg")
            nc.tensor.matmul(rb, ones1x128, rr, start=True, stop=True)
            mb = ps_lg.tile([128, 512], F32, tag="lg")
            nc.tensor.matmul(mb, ones1x128, mu, start=True, stop=True)
            tmp = st_pool.tile([128, 512], F32, tag="vtmp")
            nc.vector.tensor_sub(tmp, vraw[:, sl], mb)
            nc.vector.tensor_mul(vhat[:, sl], tmp, rb)
        for ch in range(8):
            sl = slice(ch * 512, (ch + 1) * 512)
            vp = ps_lg.tile([128, 512], F32, tag="lg")
            nc.tensor.matmul(vp, vwf_sb, vhat[:, sl], start=True, stop=True)
            nc.vector.tensor_scalar_add(V_sb[:, sl], vp, vbp_sb[:, 0:1])

        # ---------------- main loop ----------------
        srows = {}
        qins = {}

        def attn_pair(pi):
            y = 2 * pi
            qin = qin_pool.tile([64, 512], F32, tag="qin")
            nc.sync.dma_start(out=qin, in_=q_d[:, y * 256:(y + 2) * 256])
            qins[pi] = qin
            qb = qb_pool.tile([64, 512], BF16, tag="qb")
            nc.scalar.copy(qb, qin)
            xhT_ps = ps_tp.tile([64, 512], F32, tag="t")
            for c in range(4):
                csl = slice(c * 128, (c + 1) * 128)
                t1 = ps_tp.tile([128, 64], F32, tag="t")
                nc.tensor.matmul(t1, qb[:, csl], i64_sb, start=True, stop=True)
                st6 = st_pool.tile([128, 6], F32, tag="st6")
                nc.vector.bn_stats(st6, t1)
                mv = st_pool.tile([128, 2], F32, tag="mv")
                nc.vector.bn_aggr(mv, st6)
                rec = st_pool.tile([128, 1], F32, tag="rec")
                nc.vector.tensor_scalar_add(rec, mv[:, 1:2], EPS)
                nc.vector.reciprocal(rec, rec)
                rr = st_pool.tile([128, 1], F32, tag="rr")
                nc.scalar.activation(rr, rec, AF.Sqrt)
                xh = xh_pool.tile([128, 64], BF16, tag="xh")
                nc.vector.tensor_scalar(xh, t1, mv[:, 0:1], rr,
                                        ALU.subtract, ALU.mult)
                nc.tensor.matmul(xhT_ps[:, csl], xh, i128_sb,
                                 start=True, stop=True)
            xhT = xhT_pool.tile([64, 512], BF16, tag="xhT")
            nc.scalar.copy(xhT, xhT_ps)
            lg = ps_lg.tile([128, 512], F32, tag="lg")
            nc.tensor.matmul(lg, A_sb, xhT, start=True, stop=True)
            sig = sig_pool.tile([128, 512], BF16, tag="sig")
            nc.scalar.activation(sig, lg, AF.Sigmoid, bias=cb_sb[:, 0:1])
            hy = y // 4
            vsl = V_sb[:, hy * 64:(hy + 1) * 64]
            vb_ap = vsl.rearrange("p c -> p c ()").broadcast_to([128, 64, 4])
            for r in range(2):
                srow = srow_pool.tile([128, 256], BF16, tag="srow")
                nc.vector.tensor_mul(
                    srow.rearrange("p (c f) -> p c f", f=4),
                    sig[:, r * 256:(r + 1) * 256].rearrange("p (c f) -> p c f", f=4),
                    vb_ap,
                )
                srows[y + r] = srow

        def conv_block(y0):
            cv = ps_cv.tile([128, 512], F32, tag="cv")
            nc.tensor.matmul(cv, cbb_sb, ones512b, start=True, stop=False)
            for bi, dx in enumerate((1, 0, 2)):
                for ti, t in enumerate((-1, 0, 1, 2)):
                    blk = bi * 4 + ti
                    wt = cwt_sb[:, blk * 128:(blk + 1) * 128]
                    last = (dx == 2 and t == 2)
                    for p in range(2):
                        r = y0 + 2 * p + t
                        rt = srows[r] if 0 <= r <= 255 else zrow
                        base = p * 256
                        if dx == 1:
                            nc.tensor.matmul(cv[:, base:base + 256], wt,
                                             rt[:, 0:256], start=False, stop=last)
                        elif dx == 0:
                            nc.tensor.matmul(cv[:, base + 1:base + 256], wt,
                                             rt[:, 0:255], start=False, stop=last)
                        else:
                            nc.tensor.matmul(cv[:, base:base + 255], wt,
                                             rt[:, 1:256], start=False, stop=last)
            for p in range(2):
                y = y0 + 2 * p
                qin = qins.pop(y // 2)
                ot = out_pool.tile([64, 512], F32, tag="ot")
                nc.vector.tensor_add(ot[:, 0:256], cv[0:64, p * 256:(p + 1) * 256],
                                     qin[:, 0:256])
                nc.vector.tensor_add(ot[:, 256:512], cv[64:128, p * 256:(p + 1) * 256],
                                     qin[:, 256:512])
                nc.sync.dma_start(out=out_d[:, y * 256:(y + 2) * 256], in_=ot)
            for r in list(srows):
                if r < y0 + 1:
                    del srows[r]

        for pi in range(129):
            if pi < 128:
                attn_pair(pi)
            if pi >= 2 and pi % 2 == 0:
                conv_block(2 * pi - 4)

    nc.finalize()
    return nc


def _fold_weights(qW, qb, vW, vb, K, qn_g, qn_b, vn_g, vn_b, cW, cb):
    f = np.float32
    qW, qb, vW, vb, K = f(qW), f(qb), f(vW), f(vb), f(K)
    qn_g, qn_b, vn_g, vn_b, cW, cb = f(qn_g), f(qn_b), f(vn_g), f(vn_b), f(cW), f(cb)
    scale = np.float32(64.0 ** -0.5)
    qWf = qn_g[:, None] * qW.T                      # [c, co]
    bprime = qb + qW @ qn_b                         # [64]
    A = scale * (qWf @ K.T)                         # [64, 128]
    c_b = scale * (K @ bprime)                      # [128]
    vWf = vn_g[:, None] * vW.T                      # [128, 128]
    vbp = vb + vW @ vn_b                            # [128]
    cwt = np.zeros((128, 12, 128), np.float32)
    for bi, dx in enumerate((1, 0, 2)):
        for ti, t in enumerate((-1, 0, 1, 2)):
            blk = bi * 4 + ti
            if 0 <= t + 1 <= 2:
                cwt[:, blk, 0:64] = cW[:, :, t + 1, dx].T
            if 0 <= t <= 2:
                cwt[:, blk, 64:128] = cW[:, :, t, dx].T
    return {
        "Amat": np.ascontiguousarray(A.astype(NPBF16)),
        "cbias": np.ascontiguousarray(c_b.reshape(128, 1)),
        "vwf": np.ascontiguousarray(vWf.astype(NPBF16)),
        "vbp": np.ascontiguousarray(vbp.reshape(128, 1)),
        "cwt": np.ascontiguousarray(cwt.reshape(128, 1536).astype(NPBF16)),
        "cbb": np.ascontiguousarray(np.concatenate([cb, cb]).reshape(1, 128).astype(NPBF16)),
        "i64": np.ascontiguousarray(np.eye(64, dtype=np.float32).astype(NPBF16)),
        "i128": np.ascontiguousarray(np.eye(128, dtype=np.float32).astype(NPBF16)),
    }


def _run(in_maps, trace=False, **kw):
    if "nc" not in _CACHE:
        _CACHE["nc"] = _build_nc()
    return run_bass_kernel_spmd(_CACHE["nc"], in_maps, list(range(8)),
                                trace=trace, **kw)


def kernel(q, v, qW, qb, vW, vb, K, qn_g, qn_b, vn_g, vn_b, cW, cb):
    base = _fold_weights(qW, qb, vW, vb, K, qn_g, qn_b, vn_g, vn_b, cW, cb)
    in_maps = []
    for i in range(8):
        m = dict(base)
        m["q"] = np.ascontiguousarray(np.float32(q[i]).reshape(64, 65536))
        m["v"] = np.ascontiguousarray(np.float32(v[i]).reshape(128, 4096))
        in_maps.append(m)
    res = _run(in_maps)
    outs = [np.asarray(r["out"], np.float32).reshape(64, 256, 256)
            for r in res.results]
    return np.stack(outs)



# revision 32
# speedup vs baseline: 2.4137x; 2.4137x over previous
"""LocalPatchAttention Trainium2 kernel.

Data-parallel over batch B=8 across 8 NeuronCores (one image per core).
q and out live in DRAM as [128, 32768] with partitions = (channel,
row-parity): partition p<64 = channel p of even rows, p>=64 = channel p-64
of odd rows; host packs/unpacks with cheap numpy reshapes.

Per 2-row pair (128 pairs per core):
  - [128,512] q load per 2 pairs, plus a [64,512] load of the odd-row half
    so every PE contraction runs from partition base 0.
  - GPSIMD makes bf16 q and q^2 copies; PE transposes 128-px chunks into a
    shared PSUM tile whose tail columns hold per-pixel sum(q)/64 and
    sum(q^2)/64 from N=1 matmuls (no bn_stats).
  - rsqrt(var+eps) via bit-trick + 1 Newton step: shifts/int-ALU on DVE,
    multiplies on GPSIMD ([128,4] per pair, all four chunks at once).
  - normalize on DVE (2-op tensor_scalar, per-partition mean/rsqrt APs) ->
    xh bf16; PE transpose-back into xhT_ps [64,512]; Act copy to SBUF; one
    logits matmul with host-folded A = scale*(g*qW^T)K^T; Act Sigmoid
    (folded bias) split in halves; srow = sig * V on GPSIMD via stride-0
    broadcast AP.
  - 3x3 conv in fp8e4m3 with DoubleRow perf mode: srows live in a 13-slot
    fp8 ring (slot r%12, slot 12 duplicating r%12==0 rows) so each matmul
    contracts TWO vertical taps at once (12 DoubleRow matmuls per 2 pairs,
    0.5 cy/row); weights are host-scaled x32 and V by 1/32 to sit in fp8's
    normal range; K=1 bias matmul opens the PSUM group; residual q add +
    PSUM drain fused in one DVE op per pair; one [128,512] store per
    2 pairs.
Act engine only runs Copy/Identity/Sigmoid/Square -> one act table load.
V path (once per core): LN stats via 1/128-matmuls, DVE/GPSIMD rsqrt rows,
K=1 broadcast matmuls, vwf matmul, Act bias add.
"""

import numpy as np
import ml_dtypes

import concourse.bass as bass
import concourse.bacc as bacc
import concourse.tile as tile
from concourse import mybir
from concourse.bass_utils import run_bass_kernel_spmd

F32 = mybir.dt.float32
F32R = mybir.dt.float32r
BF16 = mybir.dt.bfloat16
U32 = mybir.dt.uint32
I32 = mybir.dt.int32
AF = mybir.ActivationFunctionType
ALU = mybir.AluOpType
EPS = 1e-5
MAGIC = 0x5F3759DF
NPBF16 = ml_dtypes.bfloat16

_CACHE = {}


def _build_nc():
    nc = bacc.Bacc()
    q_d = nc.declare_dram_parameter("q", [128, 32768], F32, isOutput=False)
    v_d = nc.declare_dram_parameter("v", [128, 4096], F32, isOutput=False)
    A2_d = nc.declare_dram_parameter("A2", [128, 128], BF16, isOutput=False)
    cb_d = nc.declare_dram_parameter("cbias", [128, 1], F32, isOutput=False)
    vwf_d = nc.declare_dram_parameter("vwf", [128, 128], BF16, isOutput=False)
    vbp_d = nc.declare_dram_parameter("vbp", [128, 1], F32, isOutput=False)
    cwt_d = nc.declare_dram_parameter("cwt8", [128, 1536], mybir.dt.float8e4, isOutput=False)
    cb2_d = nc.declare_dram_parameter("cb2", [128, 1], F32, isOutput=False)
    i64_d = nc.declare_dram_parameter("i64_2", [128, 64], BF16, isOutput=False)
    i128_d = nc.declare_dram_parameter("i128", [128, 128], BF16, isOutput=False)
    out_d = nc.declare_dram_parameter("out", [128, 32768], F32, isOutput=True)

    with tile.TileContext(nc) as tc, \
         tc.tile_pool(name="const", bufs=1) as cpool, \
         tc.tile_pool(name="vwork", bufs=1) as vpool, \
         tc.tile_pool(name="qin", bufs=9) as qin_pool, \
         tc.tile_pool(name="qsq", bufs=6) as qsq_pool, \
         tc.tile_pool(name="xh", bufs=8) as xh_pool, \
         tc.tile_pool(name="xhT", bufs=3) as xhT_pool, \
         tc.tile_pool(name="sig", bufs=4) as sig_pool, \
         tc.tile_pool(name="ring", bufs=1) as rg_pool, \
         tc.tile_pool(name="stat", bufs=8) as st_pool, \
         tc.tile_pool(name="vstat", bufs=2) as vst_pool, \
         tc.tile_pool(name="outp", bufs=3) as out_pool, \
         tc.tile_pool(name="ps_t1", bufs=2, space="PSUM") as ps_t1, \
         tc.tile_pool(name="ps_xt", bufs=2, space="PSUM") as ps_xt, \
         tc.tile_pool(name="ps_lg", bufs=2, space="PSUM") as ps_lg, \
         tc.tile_pool(name="ps_cv", bufs=2, space="PSUM") as ps_cv:

        def const_tile(shape, dtype, tag, src):
            t = cpool.tile(shape, dtype, tag=tag)
            nc.sync.dma_start(out=t, in_=src[:, :])
            return t

        A2_sb = const_tile([128, 128], BF16, "A2", A2_d)
        cb_sb = const_tile([128, 1], F32, "cb", cb_d)
        vwf_sb = const_tile([128, 128], BF16, "vwf", vwf_d)
        vbp_sb = const_tile([128, 1], F32, "vbp", vbp_d)
        cwt_sb = const_tile([128, 1536], mybir.dt.float8e4, "cwt", cwt_d)
        cb2_sb = const_tile([128, 1], F32, "cb2", cb2_d)
        i64_sb = const_tile([128, 64], BF16, "i64", i64_d)
        i128_sb = const_tile([128, 128], BF16, "i128", i128_d)

        w128b = cpool.tile([128, 1], BF16, tag="w128b")  # 1/64 for q sumsq
        nc.vector.memset(w128b, 1.0 / 64)
        v128b = cpool.tile([128, 1], BF16, tag="v128b")  # 1/128 for v stats
        nc.vector.memset(v128b, 1.0 / 128)
        ones1x128b = cpool.tile([1, 128], BF16, tag="o1x")
        nc.vector.memset(ones1x128b, 1.0)

        def rsqrt_pool(rr, vp, y0, t, t2):
            """rr = 1/sqrt(vp): bit trick + 1 Newton step. Scalar ALU ops on
            DVE (walrus rejects TensorScalarPtr on Pool), muls on Pool."""
            nc.vector.tensor_scalar(y0.bitcast(U32), vp.bitcast(U32), 1, None,
                                    ALU.logical_shift_right)
            nc.vector.tensor_scalar(y0.bitcast(I32), y0.bitcast(I32),
                                    -1, None, ALU.bitwise_xor)
            nc.vector.tensor_scalar(y0.bitcast(I32), y0.bitcast(I32),
                                    MAGIC + 1, None, ALU.add)
            nc.gpsimd.tensor_mul(t, y0, y0)
            nc.gpsimd.tensor_mul(t, t, vp)
            nc.vector.tensor_scalar(t2, t, -0.5, 1.5, ALU.mult, ALU.add)
            nc.gpsimd.tensor_mul(rr, y0, t2)

        # ---------------- V path (once per core) ----------------
        vraw = vpool.tile([128, 4096], F32, tag="vraw")
        vrb = vpool.tile([128, 4096], BF16, tag="vrb")
        vsq = vpool.tile([128, 4096], BF16, tag="vsq")
        vhat = vpool.tile([128, 4096], BF16, tag="vhat")
        V_sb = vpool.tile([128, 4096], F32, tag="V")
        for ch in range(8):
            sl = slice(ch * 512, (ch + 1) * 512)
            nc.sync.dma_start(out=vraw[:, sl], in_=v_d[:, sl])
            nc.gpsimd.tensor_copy(vrb[:, sl], vraw[:, sl])
            nc.scalar.activation(vsq[:, sl], vraw[:, sl], AF.Square)
            mu_ps = ps_lg.tile([1, 512], F32, tag="lg")
            nc.tensor.matmul(mu_ps, v128b, vrb[:, sl], start=True, stop=True)
            sq_ps = ps_xt.tile([1, 512], F32, tag="xt")
            nc.tensor.matmul(sq_ps, v128b, vsq[:, sl], start=True, stop=True)
            muc = vst_pool.tile([1, 512], F32, tag="vmu")
            nc.vector.tensor_copy(muc, mu_ps)
            m2 = vst_pool.tile([1, 512], F32, tag="vm2")
            nc.gpsimd.tensor_mul(m2, muc, muc)
            vpc = vst_pool.tile([1, 512], F32, tag="vvp")
            nc.vector.scalar_tensor_tensor(vpc, sq_ps, EPS, m2,
                                           ALU.add, ALU.subtract)
            ry = vst_pool.tile([1, 512], F32, tag="vry")
            rt_ = vst_pool.tile([1, 512], F32, tag="vrt")
            rt2 = vst_pool.tile([1, 512], F32, tag="vrt2")
            rrc = vst_pool.tile([1, 512], F32, tag="vrr")
            rsqrt_pool(rrc, vpc, ry, rt_, rt2)
            rrb = vst_pool.tile([1, 512], BF16, tag="vrrb")
            nc.gpsimd.tensor_copy(rrb, rrc)
            vcrb = vst_pool.tile([1, 512], BF16, tag="vcrb")
            nc.gpsimd.tensor_mul(vcrb, muc, rrc)
            rb = ps_lg.tile([128, 512], F32, tag="lg")
            nc.tensor.matmul(rb, ones1x128b, rrb, start=True, stop=True)
            cbb_ps = ps_xt.tile([128, 512], F32, tag="xt")
            nc.tensor.matmul(cbb_ps, ones1x128b, vcrb, start=True, stop=True)
            tmp = vst_pool.tile([128, 512], F32, tag="vtmp")
            nc.vector.tensor_mul(tmp, vraw[:, sl], rb)
            nc.vector.tensor_sub(vhat[:, sl], tmp, cbb_ps)
            vp_l = ps_lg.tile([128, 512], F32, tag="lg")
            nc.tensor.matmul(vp_l, vwf_sb, vhat[:, sl], start=True, stop=True)
            nc.scalar.add(V_sb[:, sl], vp_l, vbp_sb[:, 0:1])

        # ---------------- main loop ----------------
        qins = {}
        qfront = {}

        def load_2pairs(k):
            # pairs 2k, 2k+1: one [128,512] f32 load (parity-packed rows),
            # one [64,512] f32 load of the odd-row half for base-0 chunks.
            qin2 = qin_pool.tile([128, 512], F32, tag="qin")
            qinB = qin_pool.tile([64, 512], F32, tag="qinB")
            with tc.high_priority(offset=80):
                nc.sync.dma_start(out=qin2, in_=q_d[:, k * 512:(k + 1) * 512])
                nc.sync.dma_start(out=qinB,
                                  in_=q_d[64:128, k * 512:(k + 1) * 512])
            qb2 = qsq_pool.tile([128, 512], BF16, tag="qb")
            nc.gpsimd.tensor_copy(qb2, qin2)
            qbB = qsq_pool.tile([64, 512], BF16, tag="qbB")
            nc.gpsimd.tensor_copy(qbB, qinB)
            qsqA = qsq_pool.tile([64, 512], BF16, tag="qsqA")
            nc.gpsimd.tensor_mul(qsqA, qb2[0:64, :], qb2[0:64, :])
            qsqB = qsq_pool.tile([64, 512], BF16, tag="qsqB")
            nc.gpsimd.tensor_mul(qsqB, qbB, qbB)
            qfront[k] = (qin2, qb2, qbB, qsqA, qsqB)

        def attn_pair(m):
            if m % 2 == 0:
                load_2pairs(m // 2)
            qin2, qb2, qbB, qsqA, qsqB = qfront[m // 2]
            cb = (m % 2) * 256
            qins[m] = qin2[:, cb:cb + 256]
            t1 = ps_t1.tile([128, 264], F32, tag="t1")
            st4 = t1[:, 256:264]
            for c in range(4):
                csl = slice(cb + (c % 2) * 128, cb + (c % 2) * 128 + 128)
                srcq = qb2[0:64, csl] if c < 2 else qbB[:, csl]
                srcs = qsqA[:, csl] if c < 2 else qsqB[:, csl]
                nc.tensor.matmul(t1[:, c * 64:(c + 1) * 64], srcq,
                                 i64_sb[0:64, :], start=True, stop=True)
                nc.tensor.matmul(st4[:, c:c + 1], srcq, w128b[0:64, :],
                                 start=True, stop=True)
                nc.tensor.matmul(st4[:, 4 + c:5 + c], srcs, w128b[0:64, :],
                                 start=True, stop=True)
            st8 = st_pool.tile([128, 8], F32, tag="st8")
            nc.vector.tensor_copy(st8, st4)
            mu = st8[:, 0:4]
            mu2 = st_pool.tile([128, 4], F32, tag="mu2")
            nc.vector.tensor_mul(mu2, mu, mu)
            vp = st_pool.tile([128, 4], F32, tag="vp")
            nc.vector.tensor_sub(vp, st8[:, 4:8], mu2)
            y0 = st_pool.tile([128, 4], F32, tag="y0")
            t_ = st_pool.tile([128, 4], F32, tag="t_")
            t2 = st_pool.tile([128, 4], F32, tag="t2")
            rr = st_pool.tile([128, 4], F32, tag="rr")
            nc.vector.tensor_scalar(y0.bitcast(U32), vp.bitcast(U32), 1, None,
                                    ALU.logical_shift_right)
            nc.vector.tensor_scalar(y0.bitcast(I32), y0.bitcast(I32),
                                    -1, None, ALU.bitwise_xor)
            nc.vector.tensor_scalar(y0.bitcast(I32), y0.bitcast(I32),
                                    MAGIC + 1, None, ALU.add)
            nc.vector.tensor_mul(t_, y0, y0)
            nc.vector.tensor_mul(t_, t_, vp)
            nc.vector.tensor_scalar(t2, t_, -0.5, 1.5, ALU.mult, ALU.add)
            nc.vector.tensor_mul(rr, y0, t2)
            xhT_ps = ps_xt.tile([64, 512], F32, tag="xt")
            for c in range(4):
                xh = xh_pool.tile([128, 64], BF16, tag="xh")
                nc.vector.tensor_scalar(xh, t1[:, c * 64:(c + 1) * 64],
                                        mu[:, c:c + 1],
                                        rr[:, c:c + 1],
                                        ALU.subtract, ALU.mult)
                nc.tensor.matmul(xhT_ps[:, c * 128:(c + 1) * 128], xh,
                                 i128_sb, start=True, stop=True)
            xhT = xhT_pool.tile([64, 512], BF16, tag="xhT")
            nc.scalar.copy(xhT, xhT_ps)
            lg = ps_lg.tile([128, 512], F32, tag="lg")
            nc.tensor.matmul(lg, A2_sb[0:64, :], xhT, start=True, stop=True)
            sig = sig_pool.tile([128, 512], BF16, tag="sig")
            nc.scalar.activation(sig[:, 0:256], lg[:, 0:256], AF.Sigmoid,
                                 bias=cb_sb[:, 0:1])
            nc.scalar.activation(sig[:, 256:512], lg[:, 256:512], AF.Sigmoid,
                                 bias=cb_sb[:, 0:1])
            hy = m // 2
            vsl = V_sb[:, hy * 64:(hy + 1) * 64]
            vb_ap = vsl.rearrange("p c -> p c ()").broadcast_to([128, 64, 4])
            for r in range(2):
                row = 2 * m + r
                sig_ap = sig[:, r * 256:(r + 1) * 256].rearrange(
                    "p (c f) -> p c f", f=4)
                slots = [row % 12] + ([12] if row % 12 == 0 else [])
                for s in slots:
                    nc.gpsimd.tensor_mul(
                        ring[:, s * 256:(s + 1) * 256].rearrange(
                            "p (c f) -> p c f", f=4),
                        sig_ap, vb_ap)

        def conv_block(y0):
            cv = ps_cv.tile([128, 512], F32, tag="cv")
            for bi, dx in enumerate((1, 0, 2)):
                for ti, t in enumerate((-1, 0, 1, 2)):
                    blk = bi * 4 + ti
                    wt = cwt_sb[:, blk * 128:(blk + 1) * 128]
                    last = (dx == 2 and t == 2)
                    first = (dx == 1 and t == -1)
                    for p in range(2):
                        r = y0 + 2 * p + t
                        rt = srows[r] if 0 <= r <= 255 else zrow
                        base = p * 256
                        if dx == 1:
                            nc.tensor.matmul(cv[:, base:base + 256], wt,
                                             rt[:, 0:256], start=first,
                                             stop=False)
                        elif dx == 0:
                            nc.tensor.matmul(cv[:, base + 1:base + 256], wt,
                                             rt[:, 0:255], start=False,
                                             stop=False)
                        else:
                            nc.tensor.matmul(cv[:, base:base + 255], wt,
                                             rt[:, 1:256], start=False,
                                             stop=last)
            k = y0 // 4
            ot = out_pool.tile([128, 512], F32, tag="ot")
            for p in range(2):
                m = y0 // 2 + p
                qin_m = qins.pop(m)
                nc.vector.scalar_tensor_tensor(
                    ot[:, p * 256:(p + 1) * 256], qin_m, cb2_sb[:, 0:1],
                    cv[:, p * 256:(p + 1) * 256], ALU.add, ALU.add)
            nc.sync.dma_start(out=out_d[:, k * 512:(k + 1) * 512], in_=ot)
            del qfront[k]

        for pi in range(131):
            if pi < 128:
                attn_pair(pi)
            if pi >= 3 and pi % 2 == 1:
                conv_block(2 * pi - 6)

    nc.finalize()
    return nc


def _fold_weights(qW, qb, vW, vb, K, qn_g, qn_b, vn_g, vn_b, cW, cb):
    f = np.float32
    qW, qb, vW, vb, K = f(qW), f(qb), f(vW), f(vb), f(K)
    qn_g, qn_b, vn_g, vn_b, cW, cb = f(qn_g), f(qn_b), f(vn_g), f(vn_b), f(cW), f(cb)
    scale = np.float32(64.0 ** -0.5)
    qWf = qn_g[:, None] * qW.T                      # [c, co]
    bprime = qb + qW @ qn_b                         # [64]
    A = scale * (qWf @ K.T)                         # [64, 128]
    c_b = scale * (K @ bprime)                      # [128]
    vWf = vn_g[:, None] * vW.T / 32.0               # [128, 128] (1/32 for fp8)
    vbp = (vb + vW @ vn_b) / 32.0                   # [128]
    cwt = np.zeros((128, 12, 128), np.float32)
    for bi, dx in enumerate((1, 0, 2)):
        for ti, t in enumerate((-1, 0, 1, 2)):
            blk = bi * 4 + ti
            if 0 <= t + 1 <= 2:
                cwt[:, blk, 0:64] = cW[:, :, t + 1, dx].T
            if 0 <= t <= 2:
                cwt[:, blk, 64:128] = cW[:, :, t, dx].T
    # fp8 DoubleRow layout: [128, 3dx, 2 tap-pairs, 2 k-tiles, 128], x32 to
    # sit in fp8e4m3's normal range (V is scaled by 1/32 to compensate).
    cwt8 = (cwt.reshape(128, 3, 2, 2, 128) * 32.0).astype(
        ml_dtypes.float8_e4m3)
    i64_2 = np.zeros((128, 64), np.float32)
    i64_2[0:64] = np.eye(64, dtype=np.float32)
    i64_2[64:128] = np.eye(64, dtype=np.float32)
    return {
        "A2": np.ascontiguousarray(
            np.concatenate([A, A], axis=0).astype(NPBF16)),
        "cbias": np.ascontiguousarray(c_b.reshape(128, 1)),
        "vwf": np.ascontiguousarray(vWf.astype(NPBF16)),
        "vbp": np.ascontiguousarray(vbp.reshape(128, 1)),
        "cwt8": np.ascontiguousarray(cwt8.reshape(128, 1536)),
        "cb2": np.ascontiguousarray(np.concatenate([cb, cb]).reshape(128, 1)),
        "i64_2": np.ascontiguousarray(i64_2.astype(NPBF16)),
        "i128": np.ascontiguousarray(np.eye(128, dtype=np.float32).astype(NPBF16)),
    }


def _pack_q(qi):
    """[64,256,256] f32 -> [128,32768]: partitions (ch, row-parity)."""
    qs = np.empty((128, 128, 256), np.float32)
    qs[0:64] = qi[:, 0::2, :]
    qs[64:128] = qi[:, 1::2, :]
    return np.ascontiguousarray(qs.reshape(128, 32768))


def _unpack_out(r):
    """[128,32768] -> [64,256,256] undoing the row-parity packing."""
    arr = np.asarray(r, np.float32).reshape(128, 128, 256)
    out = np.empty((64, 256, 256), np.float32)
    out[:, 0::2, :] = arr[0:64]
    out[:, 1::2, :] = arr[64:128]
    return out


def _run(in_maps, trace=False, **kw):
    if "nc" not in _CACHE:
        _CACHE["nc"] = _build_nc()
    return run_bass_kernel_spmd(_CACHE["nc"], in_maps, list(range(8)),
                                trace=trace, **kw)


def kernel(q, v, qW, qb, vW, vb, K, qn_g, qn_b, vn_g, vn_b, cW, cb):
    base = _fold_weights(qW, qb, vW, vb, K, qn_g, qn_b, vn_g, vn_b, cW, cb)
    in_maps = []
    for i in range(8):
        m = dict(base)
        m["q"] = _pack_q(np.float32(q[i]))
        m["v"] = np.ascontiguousarray(np.float32(v[i]).reshape(128, 4096))
        in_maps.append(m)
    res = _run(in_maps)
    outs = [_unpack_out(r["out"]) for r in res.results]
    return np.stack(outs)
